# revision 1
# baseline (speedup 1.0000x reference)
"""DeepGEMM-style fp8 linear on 8 TRN2 NeuronCores.

Computes: out = bf16( fp8(x_pad) @ (fp8(W) * block_scale).T ) + bias, sliced to
[16384, 4000], matching the jax reference (block scales are ones, bias zeros).

Strategy: batch-parallel SPMD with HOST-side fp8 quantization. Each core gets
a 2048-row batch shard of x, pre-quantized to fp8_e4m3 and transposed to
[k, b] on host, plus the full weight pre-quantized and transposed to [k, n]
blocks. fp8 quantization on host is bit-identical to the reference's
float8_e4m3fn round-trip for this value range (verified: e4m3 and e4m3fn
encodings coincide below the e4m3 max). On device: stream fp8 tiles, fp8
matmul with DoubleRow perf mode accumulating in fp32 PSUM, add bias + cast
to bf16, store out as [n, b]; host transposes/concats the shards back.

Why: with f32 inputs the kernel moves 117MB/core (DMA-floor ~330-350us);
with fp8 inputs it moves 42MB/core (~120us), making the kernel PE-bound.
HW-measured fp8 DoubleRow runs ~190ns per [128,512] matmul (0.85 cyc/col
incl. the per-matmul LDWEIGHTS, which walrus emits per instruction with
ldw-opt disabled) => ~390us of PE for the 2048-matmul schedule; the x-load
head bubble (~24us, DMA-paced) is absorbed by a ramp phase that computes
first-half-K partials for the first ramp_nt n-tiles while x streams in.
Measured 361-483us per exec depending on device contention (baseline 628us).
PSUM matmul output is ISA-capped at 512 f32 (one bank) => bg=512, 4 groups,
8-bank rotation. DoubleRowSwInterleave mis-computes with this weight layout
and is slower in-situ; plain DoubleRow is the right mode.
"""

import sys

if "/opt/trn_rl_repo" not in sys.path:
    sys.path.insert(0, "/opt/trn_rl_repo")

import numpy as np
import ml_dtypes

P = 128
N_CORES = 8
BATCH = 16384
IN_F = 4000
OUT_F = 4000
K_PAD = 4096               # in-features padded to 32 k-subtiles of 128
N_PAD = 4096               # out-features padded 4032 -> 4096 (uniform n-tiles)

_kernel_cache = {}

# test.py knobs
TRACE = False
LAST_RESULTS = None


def _build(b_sh, ks, nt, bg, reps=1, xg=4, ramp_nt=3, wq_bufs=3, out_bufs=2,
           out_ring="sync", epi_split=False, probe="", unroll=1, pmode="dr"):
    """probe: '' normal kernel; 'pe' = x+w0 hoisted out of the reps loop,
    matmuls+epilogue only (pure PE rate); 'pe+w' = x hoisted, w streamed;
    'dma' = DMAs only, no compute (pure DMA rate).
    unroll: python-level body repetition (for TimelineSim, which cannot
    resolve For_i branches)."""
    import contextlib
    from concourse import bacc, tile, mybir
    from concourse.mybir import dt

    nbg = b_sh // bg
    assert nbg * bg == b_sh
    nxg = ks // xg
    assert nxg * xg == ks
    kk = ks // 2                      # DoubleRow k-pairs
    nc = bacc.Bacc(None, target_bir_lowering=False, debug=False)

    with tile.TileContext(nc) as tc:
        with tc.tile_pool(name="dram", bufs=1, space="DRAM") as dram:
            xt = dram.tile([nxg, P, xg, b_sh], dt.float8e4, kind="ExternalInput",
                           name="xt", uniquify=False)
            wp = dram.tile([nt, P, ks, P], dt.float8e4, kind="ExternalInput",
                           name="wp", uniquify=False)
            bvec = dram.tile([P, nt], dt.bfloat16, kind="ExternalInput",
                             name="bvec", uniquify=False)
            out = dram.tile([nt, P, b_sh], dt.bfloat16, kind="ExternalOutput",
                            name="out", uniquify=False)

        with tc.tile_pool(name="const", bufs=1) as const, \
             tc.tile_pool(name="xqp", bufs=(2 if unroll > 1 else 1)) as xqp, \
             tc.tile_pool(name="wqp", bufs=wq_bufs) as wqp, \
             tc.tile_pool(name="prtp", bufs=max(ramp_nt, 1)) as prtp, \
             tc.tile_pool(name="outp", bufs=out_bufs) as outp, \
             tc.tile_pool(name="psp",
                          bufs=max(2, min(8, (8 * 512) // max(bg, 512))),
                          space="PSUM") as psp:

            def load_bias():
                bias_bf = const.tile([P, nt], dt.bfloat16)
                nc.sync.dma_start(out=bias_bf[:, :], in_=bvec[:, :])
                bias_sb = const.tile([P, nt], dt.float32)
                nc.vector.tensor_copy(bias_sb[:, :], bias_bf[:, :])
                return bias_sb

            def load_x(xq):
                # stream fp8 straight into the resident tile, xg k-subtiles
                # per DMA (xg*b_sh contiguous bytes per partition)
                for g in range(nxg):
                    nc.sync.dma_start(out=xq[:, g * xg:(g + 1) * xg, :],
                                      in_=xt[g])

            def load_w(n):
                # weight n-tile: fp8 [P, ks, P] (4KB/partition contiguous) on
                # scalar's HWDGE ring so w loads don't queue behind x loads.
                wq = wqp.tile([P, ks, P], dt.float8e4, name="wq")
                nc.scalar.dma_start(out=wq[:, :, :], in_=wp[n])
                return wq

            xq = xqp.tile([P, ks, b_sh], dt.float8e4)

            pm = (mybir.MatmulPerfMode.DoubleRow if pmode == "dr"
                  else mybir.MatmulPerfMode.DoubleRowSwInterleave)

            def mm(wq, ps, g, k, start, stop):
                nc.tensor.matmul(
                    ps[:, :],
                    lhsT=wq[:, 2 * k:2 * k + 2, :],
                    rhs=xq[:, 2 * k:2 * k + 2, g * bg:(g + 1) * bg],
                    start=start, stop=stop,
                    perf_mode=pm)

            def ldw(wq, k):
                # explicit stationary load; probes whether walrus pairs it
                # with the following matmuls instead of re-loading per MM
                nc.tensor.ldweights(wq[:, 2 * k:2 * k + 2, :], perf_mode=pm)

            hoisted = probe in ("pe", "pe+w", "pe0", "peld")
            bias_sb = load_bias()
            wq0 = None
            if hoisted:
                load_x(xq)
                if probe in ("pe", "pe0", "peld"):
                    wq0 = load_w(0)

            with (tc.For_i(0, reps, 1) if reps > 1
                  else contextlib.nullcontext()):
              for _rep in range(unroll):
                if not hoisted:
                    load_x(xq)

                if probe == "dma":
                    # DMAs only: x (above) + w stream + out stores
                    junk = const.tile([P, b_sh], dt.bfloat16, name="junk")
                    nc.vector.memzero(junk[:, :])
                    for n in range(nt):
                        load_w(n)
                        ring = nc.sync if out_ring == "sync" else nc.scalar
                        ring.dma_start(out=out[n], in_=junk[:, :])
                else:
                    # Ramp phase: while x streams in, run first-half-K
                    # accumulation for the first ramp_nt n-tiles (uses only
                    # the first half of x); partials park in SBUF f32.
                    half = kk // 2
                    ramp_wq, ramp_part = {}, {}
                    for n in range(ramp_nt):
                        rwq = load_w(n)
                        ramp_wq[n] = rwq
                        part = prtp.tile([P, b_sh], dt.float32, name="part")
                        ramp_part[n] = part
                        pss = [psp.tile([P, bg], mybir.dt.float32, name="ps")
                               for _ in range(nbg)]
                        for k in range(half):
                            for g in range(nbg):
                                mm(rwq, pss[g], g, k, k == 0, k == half - 1)
                        for g in range(nbg):
                            nc.vector.tensor_copy(
                                part[:, g * bg:(g + 1) * bg], pss[g][:, :])

                    for n in range(nt):
                        ramp = n < ramp_nt
                        if probe in ("pe", "pe0", "peld"):
                            wq = wq0
                        elif ramp:
                            wq = ramp_wq[n]
                        else:
                            wq = load_w(n)

                        out_sb = outp.tile([P, b_sh], dt.bfloat16,
                                           name="out_sb")
                        k_lo = half if ramp else 0

                        pss = [psp.tile([P, bg], mybir.dt.float32, name="ps")
                               for _ in range(nbg)]
                        for k in range(k_lo, kk):
                            if probe == "peld":
                                ldw(wq, k)
                            for g in range(nbg):
                                mm(wq, pss[g], g, k, k == k_lo, k == kk - 1)
                        if probe == "pe0" and n != nt - 1:
                            continue
                        for g in range(nbg):
                            dst = out_sb[:, g * bg:(g + 1) * bg]
                            eng = nc.scalar if (epi_split and g % 2) \
                                else nc.vector
                            if ramp:
                                # (psum + bias) + first-half partial -> bf16
                                eng.scalar_tensor_tensor(
                                    dst, pss[g][:, :], bias_sb[:, n:n + 1],
                                    ramp_part[n][:, g * bg:(g + 1) * bg],
                                    mybir.AluOpType.add, mybir.AluOpType.add)
                            else:
                                eng.tensor_scalar_add(dst, pss[g][:, :],
                                                      bias_sb[:, n:n + 1])

                        if probe not in ("pe", "pe+w", "peld") or n == nt - 1:
                            ring = nc.sync if out_ring == "sync" \
                                else nc.scalar
                            ring.dma_start(out=out[n], in_=out_sb[:, :])

    nc.finalize()
    return nc


def make_key(reps=1):
    b_sh = BATCH // N_CORES
    return (b_sh, K_PAD // P, N_PAD // P, 512, reps)


def _get_nc(key):
    if key not in _kernel_cache:
        _kernel_cache[key] = _build(*key)
    return _kernel_cache[key]


def kernel(x, weight, weight_scale, bias):
    global LAST_RESULTS
    from concourse.bass_utils import run_bass_kernel_spmd

    x = np.asarray(x, dtype=np.float32)
    weight = np.asarray(weight, dtype=np.float32)
    weight_scale = np.asarray(weight_scale, dtype=np.float32)
    bias = np.asarray(bias)  # bf16

    n_out, k_pad = weight.shape          # 4032, 4096
    batch, in_f = x.shape                # 16384, 4000
    assert k_pad == K_PAD and batch == BATCH

    b_sh = batch // N_CORES
    ks = K_PAD // P
    nt = N_PAD // P
    xg = 4
    f8 = ml_dtypes.float8_e4m3

    # Quantize weight on host, exactly as the reference does; fold non-one
    # block scales in post-quantization (exact for power-of-two scales).
    wq8 = weight.astype(ml_dtypes.float8_e4m3fn)
    if not np.allclose(weight_scale, 1.0):
        ws = np.repeat(np.repeat(weight_scale, P, axis=0), P, axis=1)
        wq8 = (wq8.astype(np.float32) * ws[:n_out, :k_pad]).astype(
            ml_dtypes.float8_e4m3fn)
    wpad = np.zeros((N_PAD, K_PAD), dtype=f8)
    wpad[:n_out] = wq8.view(np.uint8).view(f8)
    # w -> [nt, p, ks, j]: element = w[nt*128 + j, ks*128 + p], zero-pad rows
    wp = np.ascontiguousarray(wpad.reshape(nt, P, ks, P).transpose(0, 3, 2, 1))

    # x: quantize once, pad features to K_PAD
    xq8 = np.zeros((batch, K_PAD), dtype=f8)
    xq8[:, :in_f] = x.astype(ml_dtypes.float8_e4m3fn).view(np.uint8).view(f8)

    # bias -> [p, nt] bf16, zero-padded
    bpad = np.zeros(N_PAD, dtype=ml_dtypes.bfloat16)
    bpad[:n_out] = bias
    bvec = np.ascontiguousarray(bpad.reshape(nt, P).T)

    in_maps = []
    for c in range(N_CORES):
        shard = xq8[c * b_sh:(c + 1) * b_sh]        # [b_sh, K_PAD] fp8
        # -> [nxg, P, xg, b_sh]: element (g,p,j,b) = x[b, (g*xg+j)*P + p]
        xt = np.ascontiguousarray(
            shard.T.reshape(ks // xg, xg, P, b_sh).transpose(0, 2, 1, 3))
        in_maps.append({"xt": xt, "wp": wp, "bvec": bvec})

    global _last_in_maps
    _last_in_maps = in_maps
    nc = _get_nc(make_key(1))
    res = run_bass_kernel_spmd(nc, in_maps, list(range(N_CORES)), trace=TRACE)
    LAST_RESULTS = res

    final = np.empty((batch, OUT_F), dtype=ml_dtypes.bfloat16)
    for c in range(N_CORES):
        oc = res.results[c]["out"].reshape(N_PAD, b_sh)
        final[c * b_sh:(c + 1) * b_sh, :] = oc[:OUT_F].T
    return final

